# revision 7
# baseline (speedup 1.0000x reference)
"""AtomicConvolution Trainium2 kernel (8 NeuronCores, SPMD, no collectives).

Sharding: N-shard. Core r handles atoms [256r, 256r+256) for ALL 16 batches.
The X coordinate table (tiny) is replicated per core, so the neighbor gather
is core-local and the batch-norm moments over axis 0 (batch) are core-local
too (each core holds all 16 batches for its atoms). No cross-core traffic.

Per-core layout:
  - partition-group g (16 partitions, g=0..7) owns batches {2g, 2g+1}
  - gather table tbl[16g+c, beta*2048 + j] = plane c of X[2g+beta, j]
    (c=0,1,2 -> x,y,z; c=3 -> |x|^2 computed on device)
  - ap_gather (d=1) pulls all 16 channels per index -> x,y,z,q gathered at once
  - centers ch3 = 1.0, so prod = gather * centers has q_j at ch3 and
    x_j*x_n at ch0..2; R^2 = one matmul with weights (-2,-2,-2,+1) + one
    accumulating K=8 matmul adding q_n
  - compacted pair layout [128, 2048]: partition p = 16g + 8beta + nb,
    free = (ns, m), atom n_loc = 32 nb + ns  (b = 2g+beta)
  - rsf_l = exp(-re(R-rs)^2) * 0.5*(cos(pi R/rc)+1) * [R<=rc]:
      u = R^2 - 2 rs R  (DVE stt); K' = Exp(-re*u - re*rs^2 + ln 0.5)  (ACT)
      FCx2 = 1 + Sin(3pi/2 - Relu(pi - pi R/rc))  (exact cutoff, clamped arg)
      rsf = (sin_out + 1) * K'  (DVE stt)
  - sym[n, a*12+l] = sum_m rsf_l * (Z==type_a): bf16 mult + segmented reduce
  - BN over the 16 batches: PE stride-8 partition reduction + broadcast.
"""
import sys

if '/opt/trn_rl_repo' not in sys.path:
    sys.path.insert(0, '/opt/trn_rl_repo')

import math
import numpy as np

import concourse.bacc as bacc
import concourse.bass as bass
import concourse.mybir as mybir
from concourse import library_config
from concourse.tile import TileContext

F32 = mybir.dt.float32
BF16 = mybir.dt.bfloat16
I16 = mybir.dt.int16
AF = mybir.ActivationFunctionType
ALU = mybir.AluOpType

P = 128
B, N, M, L, A = 16, 2048, 64, 12, 4
NSH = N // 8                 # atoms per core = 256
NCHUNK = 8                   # gather chunks per core
CI = 4096                    # indices per group per chunk
TF = 2048                    # compacted R free size (= 32 ns * 64 m)
NFEAT = A * L                # 48
OUTF = 32 * NFEAT            # 1536 output cols per partition
ATOM_TYPES = (1, 6, 7, 8)
BN_EPS = 1e-3
PI = math.pi


def build_nc(rc_v, rs_v, re_v):
    """Build the per-core graph. rc/rs/re are baked in as immediates."""
    rc_v = [float(x) for x in rc_v]
    rs_v = [float(x) for x in rs_v]
    re_v = [float(x) for x in re_v]
    rc_groups = {}
    for l, v in enumerate(rc_v):
        rc_groups.setdefault(v, []).append(l)
    rc_list = list(rc_groups.keys())
    rcg_of_l = {}
    for gi, v in enumerate(rc_list):
        for l in rc_groups[v]:
            rcg_of_l[l] = gi

    nc = bacc.Bacc()
    tbl_in = nc.declare_dram_parameter("tbl", [P, 2 * N], F32, isOutput=False)
    gidx_in = nc.declare_dram_parameter("gidx", [P, TF], I16, isOutput=False)
    cen_in = nc.declare_dram_parameter("cen", [P, 2 * NSH], F32, isOutput=False)
    zc_in = nc.declare_dram_parameter("zc", [P, TF], F32, isOutput=False)
    w1_in = nc.declare_dram_parameter("w1", [P, 8], F32, isOutput=False)
    wq_in = nc.declare_dram_parameter("wq", [P, 8], F32, isOutput=False)
    id8_in = nc.declare_dram_parameter("id8", [8, 8], F32, isOutput=False)
    bnred_in = nc.declare_dram_parameter("bnred", [P, 8], F32, isOutput=False)
    bnbc_in = nc.declare_dram_parameter("bnbc", [8, P], F32, isOutput=False)
    cb_in = nc.declare_dram_parameter("cbias", [P, 16], F32, isOutput=False)
    out_ext = nc.declare_dram_parameter("out", [P, OUTF], F32, isOutput=True)

    rr_dram = nc.dram_tensor("rr", [NCHUNK, 8, CI], F32)

    with TileContext(nc) as tc:
        with tc.tile_pool(name="sbuf", bufs=1) as pool, \
             tc.tile_pool(name="psum", bufs=1, space="PSUM") as psum:
            nc.gpsimd.load_library(library_config.ap_gather)

            tbl = pool.tile([P, 2 * N], F32)
            gidx = pool.tile([P, TF], I16)
            cen = pool.tile([P, 2 * NSH], F32)
            zc = pool.tile([P, TF], F32)
            w1 = pool.tile([P, 8], F32)
            wq = pool.tile([P, 8], F32)
            id8 = pool.tile([8, 8], F32)
            bnred = pool.tile([P, 8], F32)
            bnbc = pool.tile([8, P], F32)
            cb = pool.tile([P, 16], F32)
            for t, src in [(tbl, tbl_in), (gidx, gidx_in), (cen, cen_in),
                           (zc, zc_in), (w1, w1_in), (wq, wq_in), (id8, id8_in),
                           (bnred, bnred_in), (bnbc, bnbc_in), (cb, cb_in)]:
                nc.sync.dma_start(out=t[:], in_=src[:])

            # ---- q-plane prep: tbl channel 16g+3 <- |x_j|^2
            sqt = pool.tile([P, 2 * N], F32, tag="prod", bufs=2)
            nc.vector.tensor_tensor(out=sqt[:], in0=tbl[:], in1=tbl[:], op=ALU.mult)
            qsb = pool.tile([8, 2 * N], F32, tag="rsp", bufs=2)
            for h in range(4):
                pq = psum.tile([8, 1024], F32, tag="pchunk", bufs=2)
                for j in range(2):
                    c0 = 1024 * h + 512 * j
                    nc.tensor.matmul(out=pq[:, 512 * j:512 * j + 512], lhsT=wq[:],
                                     rhs=sqt[:, c0:c0 + 512], start=True, stop=True)
                nc.scalar.activation(out=qsb[0:8, 1024 * h:1024 * h + 1024],
                                     in_=pq[:], func=AF.Copy)
            nc.sync.dma_start(out=tbl[3:116:16, :], in_=qsb[0:8, :])

            # ---- q-plane for centers (kept at partitions 0..7)
            sqc = pool.tile([P, 2 * NSH], F32)
            nc.vector.tensor_tensor(out=sqc[:], in0=cen[:], in1=cen[:], op=ALU.mult)
            qcen = pool.tile([8, 2 * NSH], F32)
            pqc = psum.tile([8, 512], F32, tag="pbn0")
            nc.tensor.matmul(out=pqc[:], lhsT=wq[:], rhs=sqc[:], start=True, stop=True)
            nc.scalar.activation(out=qcen[0:8, :], in_=pqc[:], func=AF.Copy)

            sym = pool.tile([P, OUTF], F32)
            Rt = pool.tile([P, TF], F32)
            cen_pitch = cen[:].ap[0][0]
            qcen_pitch = qcen[:].ap[0][0]

            for k in range(NCHUNK):
                # ---- gather chunk k: 4096 idx per group
                gch = pool.tile([P, CI], F32, tag="gch", bufs=2)
                nc.gpsimd.ap_gather(
                    out_ap=gch[:], in_ap=tbl[:],
                    idxs_ap=gidx[:, 256 * k:256 * (k + 1)],
                    channels=P, num_elems=2 * N, d=1, num_idxs=CI)

                # ---- products vs centers; ch3: q_j * 1.0 passes through
                prod = pool.tile([P, CI], F32, tag="prod", bufs=2)
                for beta in range(2):
                    cen_ap = bass.AP(
                        cen.tensor,
                        cen[:].offset + 256 * beta + 4 * k,
                        [[cen_pitch, P], [32, 8], [1, 4], [0, 64]])
                    nc.vector.tensor_tensor(
                        out=prod[:, 2048 * beta:2048 * (beta + 1)],
                        in0=gch[:, 2048 * beta:2048 * (beta + 1)],
                        in1=cen_ap, op=ALU.mult)

                # ---- R^2 via PE, then sqrt
                rsp = pool.tile([8, CI], F32, tag="rsp", bufs=2)
                for h in range(4):
                    ps = psum.tile([8, 1024], F32, tag="pchunk", bufs=2)
                    for j in range(2):
                        c0 = 1024 * h + 512 * j
                        beta = c0 // 2048
                        nb0 = (c0 % 2048) // 256
                        nc.tensor.matmul(out=ps[:, 512 * j:512 * j + 512],
                                         lhsT=w1[:], rhs=prod[:, c0:c0 + 512],
                                         start=True, stop=False)
                        qn_ap = bass.AP(
                            qcen.tensor,
                            qcen[:].offset + 256 * beta + 32 * nb0 + 4 * k,
                            [[qcen_pitch, 8], [32, 2], [1, 4], [0, 64]])
                        nc.tensor.matmul(out=ps[:, 512 * j:512 * j + 512],
                                         lhsT=id8[:], rhs=qn_ap,
                                         start=False, stop=True)
                    nc.scalar.activation(out=rsp[0:8, 1024 * h:1024 * h + 1024],
                                         in_=ps[:], func=AF.Sqrt,
                                         bias=cb[0:8, 15:16])
                # round-trip compaction [8, 4096] -> [128, 256]
                nc.sync.dma_start(out=rr_dram[k], in_=rsp[0:8, :])
                nc.sync.dma_start(
                    out=Rt[:, 256 * k:256 * (k + 1)],
                    in_=rr_dram[k].rearrange("g (p f) -> (g p) f", p=16))

                if k % 2 == 1:
                    _quarter(nc, pool, psum, Rt, zc, sym, bnred, bnbc, cb,
                             out_ext, k // 2, rc_list, rcg_of_l, rs_v, re_v)
    nc.compile()
    return nc


def _quarter(nc, pool, psum, Rt, zc, sym, bnred, bnbc, cb, out_ext,
             q, rc_list, rcg_of_l, rs_v, re_v):
    """rsf + masked reduce + BN for R columns [512q, 512q+512)."""
    fsl = slice(512 * q, 512 * (q + 1))
    rsq = pool.tile([P, 512], F32, tag="rsq", bufs=2)
    nc.vector.tensor_tensor(out=rsq[:], in0=Rt[:, fsl], in1=Rt[:, fsl], op=ALU.mult)

    c1s = []
    for gi, rcval in enumerate(rc_list):
        ur = pool.tile([P, 512], F32, tag="ur", bufs=2)
        nc.scalar.activation(out=ur[:], in_=Rt[:, fsl], func=AF.Relu,
                             scale=-PI / rcval, bias=cb[:, 0:1])
        c1 = pool.tile([P, 512], BF16, tag=f"c1_{gi}")
        nc.scalar.activation(out=c1[:], in_=ur[:], func=AF.Sin,
                             scale=-1.0, bias=cb[:, 1:2])  # sin(pi/2 - u) = cos(u)
        c1s.append(c1)

    masks = []
    for a in range(A):
        mk = pool.tile([P, 512], BF16, tag=f"mask_{a}")
        nc.vector.tensor_scalar(out=mk[:], in0=zc[:, fsl],
                                scalar1=float(ATOM_TYPES[a]), scalar2=None,
                                op0=ALU.is_equal)
        masks.append(mk)

    for l in range(L):
        u = pool.tile([P, 512], F32, tag="u", bufs=2)
        nc.vector.scalar_tensor_tensor(
            out=u[:], in0=Rt[:, fsl], scalar=-2.0 * rs_v[l], in1=rsq[:],
            op0=ALU.mult, op1=ALU.add)
        kp = pool.tile([P, 512], BF16, tag="kp", bufs=2)
        nc.scalar.activation(out=kp[:], in_=u[:], func=AF.Exp,
                             scale=-re_v[l], bias=cb[:, 3 + l:4 + l])
        rsf = pool.tile([P, 512], BF16, tag="rsf", bufs=2)
        nc.vector.scalar_tensor_tensor(
            out=rsf[:], in0=c1s[rcg_of_l[l]][:], scalar=1.0, in1=kp[:],
            op0=ALU.subtract, op1=ALU.mult)
        for a in range(A):
            pm = pool.tile([P, 512], BF16, tag="pm", bufs=2)
            nc.vector.tensor_tensor(out=pm[:], in0=rsf[:], in1=masks[a][:],
                                    op=ALU.mult)
            col0 = 384 * q + a * L + l
            nc.vector.tensor_reduce(
                out=sym[:, col0:col0 + 7 * NFEAT + 1:NFEAT],
                in_=pm[:].rearrange("p (s m) -> p s m", s=8),
                axis=mybir.AxisListType.X, op=ALU.add)

    # ---- batch-norm for this quarter's 384 output columns
    cf = slice(384 * q, 384 * (q + 1))
    ssq = pool.tile([P, 384], F32, tag="ssq", bufs=2)
    nc.vector.tensor_tensor(out=ssq[:], in0=sym[:, cf], in1=sym[:, cf], op=ALU.mult)
    pm1 = psum.tile([8, 384], F32, tag="pbn0")
    nc.tensor.matmul(out=pm1[:], lhsT=bnred[:], rhs=sym[:, cf], start=True, stop=True)
    pm2 = psum.tile([8, 384], F32, tag="pbn1")
    nc.tensor.matmul(out=pm2[:], lhsT=bnred[:], rhs=ssq[:], start=True, stop=True)
    msb = pool.tile([8, 384], F32, tag="msb", bufs=2)
    nc.vector.tensor_copy(out=msb[0:8, :], in_=pm1[:])
    m2 = pool.tile([8, 384], F32, tag="m2", bufs=2)
    nc.vector.tensor_tensor(out=m2[0:8, :], in0=msb[0:8, :], in1=msb[0:8, :],
                            op=ALU.mult)
    vsb = pool.tile([8, 384], F32, tag="vsb", bufs=2)
    nc.vector.tensor_tensor(out=vsb[0:8, :], in0=pm2[:], in1=m2[0:8, :],
                            op=ALU.subtract)
    ssb = pool.tile([8, 384], F32, tag="ssb", bufs=2)
    nc.scalar.activation(out=ssb[0:8, :], in_=vsb[0:8, :], func=AF.Sqrt,
                         bias=cb[0:8, 2:3])
    rsb = pool.tile([8, 384], F32, tag="rsb", bufs=2)
    nc.vector.reciprocal(out=rsb[0:8, :], in_=ssb[0:8, :])
    pbm = psum.tile([P, 384], F32, tag="pbn2")
    nc.tensor.matmul(out=pbm[:], lhsT=bnbc[:], rhs=msb[0:8, :], start=True, stop=True)
    pbr = psum.tile([P, 384], F32, tag="pbn3")
    nc.tensor.matmul(out=pbr[:], lhsT=bnbc[:], rhs=rsb[0:8, :], start=True, stop=True)
    dsb = pool.tile([P, 384], F32, tag="dsb", bufs=2)
    nc.vector.tensor_tensor(out=dsb[:], in0=pbm[:], in1=sym[:, cf], op=ALU.subtract)
    osb = pool.tile([P, 384], F32, tag="osb", bufs=2)
    nc.vector.tensor_tensor(out=osb[:], in0=dsb[:], in1=pbr[:], op=ALU.mult)
    nc.sync.dma_start(out=out_ext[:, cf], in_=osb[:])


# ---------------------------------------------------------------- host side

def make_cbias(rs_v, re_v):
    cb = np.zeros((P, 16), np.float32)
    cb[:, 0] = PI
    cb[:, 1] = 0.5 * PI
    cb[:, 2] = BN_EPS
    cb[:, 15] = 1e-4
    for l in range(L):
        cb[:, 3 + l] = -float(re_v[l]) * float(rs_v[l]) ** 2 + math.log(0.5)
    return cb


def prep_core_inputs(X, Nbrs, Nbrs_Z, r, const_cache={}):
    """Build core r's input map (numpy layout prep only)."""
    n0 = NSH * r
    Xt = np.ascontiguousarray(X.transpose(2, 0, 1))          # [3, B, N]
    if "tbl" not in const_cache:
        tbl = np.zeros((8, 16, 2, N), np.float32)
        tbl[:, 0:3, :, :] = Xt.reshape(3, 8, 2, N).transpose(1, 0, 2, 3)
        const_cache["tbl"] = tbl.reshape(P, 2 * N)

        w1 = np.zeros((P, 8), np.float32)
        wq = np.zeros((P, 8), np.float32)
        for g in range(8):
            w1[16 * g + 0:16 * g + 3, g] = -2.0
            w1[16 * g + 3, g] = 1.0
            wq[16 * g + 0:16 * g + 3, g] = 1.0
        bnred = np.zeros((P, 8), np.float32)
        bnbc = np.zeros((8, P), np.float32)
        for p in range(P):
            bnred[p, p % 8] = 1.0 / 16.0
            bnbc[p % 8, p] = 1.0
        const_cache["w1"] = w1
        const_cache["wq"] = wq
        const_cache["id8"] = np.eye(8, dtype=np.float32)
        const_cache["bnred"] = bnred
        const_cache["bnbc"] = bnbc
        const_cache["cbias"] = None  # filled by caller

    cen = np.zeros((8, 16, 2, NSH), np.float32)
    cen[:, 0:3, :, :] = (Xt[:, :, n0:n0 + NSH]
                         .reshape(3, 8, 2, NSH).transpose(1, 0, 2, 3))
    cen[:, 3, :, :] = 1.0                                     # q_j passthrough
    cen = cen.reshape(P, 2 * NSH)

    nbr_sh = Nbrs[:, n0:n0 + NSH, :]                          # [16, 256, 64]
    nbr6 = nbr_sh.reshape(8, 2, 8, 8, 4, M)                   # [g, beta, nb, k, j, m]
    lg = nbr6 + (np.arange(2, dtype=nbr6.dtype)
                 .reshape(1, 2, 1, 1, 1, 1) * N)
    lg = lg.transpose(0, 3, 1, 2, 4, 5).reshape(8, NCHUNK * CI)
    gidx = (lg.reshape(8, TF, 16).transpose(0, 2, 1)
            .reshape(P, TF).astype(np.int16))

    zc = (Nbrs_Z[:, n0:n0 + NSH, :].reshape(8, 2, 8, 32, M)
          .reshape(P, TF).astype(np.float32))

    return {"tbl": const_cache["tbl"], "gidx": gidx, "cen": cen, "zc": zc,
            "w1": const_cache["w1"], "wq": const_cache["wq"],
            "id8": const_cache["id8"], "bnred": const_cache["bnred"],
            "bnbc": const_cache["bnbc"], "cbias": const_cache["cbias"]}


def assemble_output(results):
    full = np.empty((8, 2, N, NFEAT), np.float32)             # [g, beta, n, f]
    for r in range(8):
        o = np.asarray(results[r]["out"]).reshape(8, 2, NSH, NFEAT)
        n0 = NSH * r
        full[:, :, n0:n0 + NSH, :] = o
    return full.reshape(B, N, NFEAT)


_cache = {}


def kernel(X, Nbrs, Nbrs_Z, rc, rs, re):
    from concourse.bass_utils import run_bass_kernel_spmd
    key = (tuple(np.asarray(rc).ravel().tolist()),
           tuple(np.asarray(rs).ravel().tolist()),
           tuple(np.asarray(re).ravel().tolist()))
    if key not in _cache:
        _cache[key] = build_nc(np.asarray(rc).ravel(), np.asarray(rs).ravel(),
                               np.asarray(re).ravel())
    nc = _cache[key]
    X = np.asarray(X, np.float32)
    Nbrs = np.asarray(Nbrs)
    Nbrs_Z = np.asarray(Nbrs_Z)
    cc = {}
    in_maps = [prep_core_inputs(X, Nbrs, Nbrs_Z, r, cc) for r in range(8)]
    cbias = make_cbias(np.asarray(rs).ravel(), np.asarray(re).ravel())
    for im in in_maps:
        im["cbias"] = cbias
    res = run_bass_kernel_spmd(nc, in_maps, core_ids=list(range(8)))
    return assemble_output(res.results)


# revision 9
# speedup vs baseline: 25.2168x; 25.2168x over previous
"""AtomicConvolution Trainium2 kernel (8 NeuronCores, SPMD, no collectives).

Sharding: N-shard. Core r handles atoms [256r, 256r+256) for ALL 16 batches.
The X coordinate table (tiny) is replicated per core, so the neighbor gather
is core-local and the batch-norm moments over axis 0 (batch) are core-local
too (each core holds all 16 batches for its atoms). No cross-core traffic.

Per-core layout:
  - partition-group g (16 partitions, g=0..7) owns batches {2g, 2g+1}
  - gather table tbl[16g+c, beta*2048 + j] = plane c of X[2g+beta, j]
    (c=0,1,2 -> x,y,z; c=3 -> |x|^2 computed on device)
  - ap_gather (d=1) pulls all 16 channels per index -> x,y,z,q gathered at once
  - centers ch3 = 1.0, so prod = gather * centers has q_j at ch3 and
    x_j*x_n at ch0..2; R^2 = one matmul with weights (-2,-2,-2,+1) + one
    accumulating K=8 matmul adding q_n
  - compacted pair layout [128, 2048]: partition p = 16g + 8beta + nb,
    free = (ns, m), atom n_loc = 32 nb + ns  (b = 2g+beta)
  - rsf_l = exp(-re(R-rs)^2) * 0.5*(cos(pi R/rc)+1) * [R<=rc]:
      u = R^2 - 2 rs R  (DVE stt); K' = Exp(-re*u - re*rs^2 + ln 0.5)  (ACT)
      FCx2 = 1 + Sin(3pi/2 - Relu(pi - pi R/rc))  (exact cutoff, clamped arg)
      rsf = (sin_out + 1) * K'  (DVE stt)
  - sym[n, a*12+l] = sum_m rsf_l * (Z==type_a): bf16 mult + segmented reduce
  - BN over the 16 batches: PE stride-8 partition reduction + broadcast.
"""
import sys

if '/opt/trn_rl_repo' not in sys.path:
    sys.path.insert(0, '/opt/trn_rl_repo')

import math
import numpy as np

import concourse.bacc as bacc
import concourse.bass as bass
import concourse.mybir as mybir
from concourse import library_config
from concourse.tile import TileContext

F32 = mybir.dt.float32
BF16 = mybir.dt.bfloat16
I16 = mybir.dt.int16
AF = mybir.ActivationFunctionType
ALU = mybir.AluOpType

P = 128
B, N, M, L, A = 16, 2048, 64, 12, 4
NSH = N // 8                 # atoms per core = 256
NCHUNK = 8                   # gather chunks per core
CI = 4096                    # indices per group per chunk
TF = 2048                    # compacted R free size (= 32 ns * 64 m)
NFEAT = A * L                # 48
OUTF = 32 * NFEAT            # 1536 output cols per partition
ATOM_TYPES = (1, 6, 7, 8)
BN_EPS = 1e-3
PI = math.pi


def build_nc(rc_v, rs_v, re_v, reps=None):
    """Build the per-core graph. rc/rs/re are baked in as immediates.
    reps: if set, wrap the whole body in a HW For_i loop (for benchmarking)."""
    rc_v = [float(x) for x in rc_v]
    rs_v = [float(x) for x in rs_v]
    re_v = [float(x) for x in re_v]
    rc_groups = {}
    for l, v in enumerate(rc_v):
        rc_groups.setdefault(v, []).append(l)
    rc_list = list(rc_groups.keys())
    rcg_of_l = {}
    for gi, v in enumerate(rc_list):
        for l in rc_groups[v]:
            rcg_of_l[l] = gi

    nc = bacc.Bacc()
    tbl_in = nc.declare_dram_parameter("tbl", [P, 2 * N], F32, isOutput=False)
    gidx_in = nc.declare_dram_parameter("gidx", [P, TF], I16, isOutput=False)
    cen_in = nc.declare_dram_parameter("cen", [P, 2 * NSH], F32, isOutput=False)
    zc_in = nc.declare_dram_parameter("zc", [P, TF], F32, isOutput=False)
    w1_in = nc.declare_dram_parameter("w1", [P, 8], F32, isOutput=False)
    wq_in = nc.declare_dram_parameter("wq", [P, 8], F32, isOutput=False)
    id8_in = nc.declare_dram_parameter("id8", [8, 8], F32, isOutput=False)
    bnred_in = nc.declare_dram_parameter("bnred", [P, 8], F32, isOutput=False)
    bnbc_in = nc.declare_dram_parameter("bnbc", [8, P], F32, isOutput=False)
    cb_in = nc.declare_dram_parameter("cbias", [P, 16], F32, isOutput=False)
    out_ext = nc.declare_dram_parameter("out", [P, OUTF], F32, isOutput=True)

    rr_dram = nc.dram_tensor("rr", [NCHUNK, 8, CI], F32)

    import contextlib
    with TileContext(nc) as tc:
        with tc.tile_pool(name="sbuf", bufs=1) as pool, \
             tc.tile_pool(name="psum", bufs=1, space="PSUM") as psum:
            nc.gpsimd.load_library(library_config.ap_gather)
            loop_cm = tc.For_i(0, reps, 1) if reps else contextlib.nullcontext()
            _body_build(nc, tc, pool, psum, loop_cm,
                        tbl_in, gidx_in, cen_in, zc_in, w1_in, wq_in, id8_in,
                        bnred_in, bnbc_in, cb_in, out_ext, rr_dram,
                        rc_list, rcg_of_l, rs_v, re_v)
    nc.compile()
    return nc


def _body_build(nc, tc, pool, psum, loop_cm,
                tbl_in, gidx_in, cen_in, zc_in, w1_in, wq_in, id8_in,
                bnred_in, bnbc_in, cb_in, out_ext, rr_dram,
                rc_list, rcg_of_l, rs_v, re_v):
    with loop_cm:
            tbl = pool.tile([P, 2 * N], F32)
            gidx = pool.tile([P, TF], I16)
            cen = pool.tile([P, 2 * NSH], F32)
            zc = pool.tile([P, TF], F32)
            w1 = pool.tile([P, 8], F32)
            wq = pool.tile([P, 8], F32)
            id8 = pool.tile([8, 8], F32)
            bnred = pool.tile([P, 8], F32)
            bnbc = pool.tile([8, P], F32)
            cb = pool.tile([P, 16], F32)
            for t, src in [(tbl, tbl_in), (gidx, gidx_in), (cen, cen_in),
                           (zc, zc_in), (w1, w1_in), (wq, wq_in), (id8, id8_in),
                           (bnred, bnred_in), (bnbc, bnbc_in), (cb, cb_in)]:
                nc.sync.dma_start(out=t[:], in_=src[:])

            # ---- q-plane prep: tbl channel 16g+3 <- |x_j|^2
            sqt = pool.tile([P, 2 * N], F32, tag="prod", bufs=2)
            nc.vector.tensor_tensor(out=sqt[:], in0=tbl[:], in1=tbl[:], op=ALU.mult)
            qsb = pool.tile([8, 2 * N], F32, tag="rsp", bufs=2)
            for h in range(4):
                pq = psum.tile([8, 1024], F32, tag="pchunk", bufs=2)
                for j in range(2):
                    c0 = 1024 * h + 512 * j
                    nc.tensor.matmul(out=pq[:, 512 * j:512 * j + 512], lhsT=wq[:],
                                     rhs=sqt[:, c0:c0 + 512], start=True, stop=True)
                nc.scalar.activation(out=qsb[0:8, 1024 * h:1024 * h + 1024],
                                     in_=pq[:], func=AF.Copy)
            nc.sync.dma_start(out=tbl[3:116:16, :], in_=qsb[0:8, :])

            # ---- q-plane for centers (kept at partitions 0..7)
            sqc = pool.tile([P, 2 * NSH], F32)
            nc.vector.tensor_tensor(out=sqc[:], in0=cen[:], in1=cen[:], op=ALU.mult)
            qcen = pool.tile([8, 2 * NSH], F32)
            pqc = psum.tile([8, 512], F32, tag="pbn0")
            nc.tensor.matmul(out=pqc[:], lhsT=wq[:], rhs=sqc[:], start=True, stop=True)
            nc.scalar.activation(out=qcen[0:8, :], in_=pqc[:], func=AF.Copy)

            sym = pool.tile([P, OUTF], F32)
            Rt = pool.tile([P, TF], F32)
            cen_pitch = cen[:].ap[0][0]
            qcen_pitch = qcen[:].ap[0][0]

            for k in range(NCHUNK):
                # ---- gather chunk k: 4096 idx per group
                gch = pool.tile([P, CI], F32, tag="gch", bufs=2)
                nc.gpsimd.ap_gather(
                    out_ap=gch[:], in_ap=tbl[:],
                    idxs_ap=gidx[:, 256 * k:256 * (k + 1)],
                    channels=P, num_elems=2 * N, d=1, num_idxs=CI)

                # ---- products vs centers; ch3: q_j * 1.0 passes through
                prod = pool.tile([P, CI], F32, tag="prod", bufs=2)
                for beta in range(2):
                    cen_ap = bass.AP(
                        cen.tensor,
                        cen[:].offset + 256 * beta + 4 * k,
                        [[cen_pitch, P], [32, 8], [1, 4], [0, 64]])
                    nc.vector.tensor_tensor(
                        out=prod[:, 2048 * beta:2048 * (beta + 1)],
                        in0=gch[:, 2048 * beta:2048 * (beta + 1)],
                        in1=cen_ap, op=ALU.mult)

                # ---- R^2 via PE, then sqrt
                rsp = pool.tile([8, CI], F32, tag="rsp", bufs=2)
                for h in range(4):
                    ps = psum.tile([8, 1024], F32, tag="pchunk", bufs=2)
                    for j in range(2):
                        c0 = 1024 * h + 512 * j
                        beta = c0 // 2048
                        nb0 = (c0 % 2048) // 256
                        nc.tensor.matmul(out=ps[:, 512 * j:512 * j + 512],
                                         lhsT=w1[:], rhs=prod[:, c0:c0 + 512],
                                         start=True, stop=False)
                        qn_ap = bass.AP(
                            qcen.tensor,
                            qcen[:].offset + 256 * beta + 32 * nb0 + 4 * k,
                            [[qcen_pitch, 8], [32, 2], [1, 4], [0, 64]])
                        nc.tensor.matmul(out=ps[:, 512 * j:512 * j + 512],
                                         lhsT=id8[:], rhs=qn_ap,
                                         start=False, stop=True)
                    nc.scalar.activation(out=rsp[0:8, 1024 * h:1024 * h + 1024],
                                         in_=ps[:], func=AF.Sqrt,
                                         bias=cb[0:8, 15:16])
                # round-trip compaction [8, 4096] -> [128, 256]
                nc.sync.dma_start(out=rr_dram[k], in_=rsp[0:8, :])
                nc.sync.dma_start(
                    out=Rt[:, 256 * k:256 * (k + 1)],
                    in_=rr_dram[k].rearrange("g (p f) -> (g p) f", p=16))

                if k % 2 == 1:
                    _quarter(nc, pool, psum, Rt, zc, sym, bnred, bnbc, cb,
                             out_ext, k // 2, rc_list, rcg_of_l, rs_v, re_v)


def _quarter(nc, pool, psum, Rt, zc, sym, bnred, bnbc, cb, out_ext,
             q, rc_list, rcg_of_l, rs_v, re_v):
    """rsf + masked reduce + BN for R columns [512q, 512q+512)."""
    fsl = slice(512 * q, 512 * (q + 1))
    rsq = pool.tile([P, 512], F32, tag="rsq", bufs=2)
    nc.vector.tensor_tensor(out=rsq[:], in0=Rt[:, fsl], in1=Rt[:, fsl], op=ALU.mult)

    c1s = []
    for gi, rcval in enumerate(rc_list):
        ur = pool.tile([P, 512], F32, tag="ur", bufs=2)
        nc.scalar.activation(out=ur[:], in_=Rt[:, fsl], func=AF.Relu,
                             scale=-PI / rcval, bias=cb[:, 0:1])
        c1 = pool.tile([P, 512], BF16, tag=f"c1_{gi}")
        nc.scalar.activation(out=c1[:], in_=ur[:], func=AF.Sin,
                             scale=-1.0, bias=cb[:, 1:2])  # sin(pi/2 - u) = cos(u)
        c1s.append(c1)

    masks = []
    for a in range(A):
        mk = pool.tile([P, 512], BF16, tag=f"mask_{a}")
        nc.vector.tensor_scalar(out=mk[:], in0=zc[:, fsl],
                                scalar1=float(ATOM_TYPES[a]), scalar2=None,
                                op0=ALU.is_equal)
        masks.append(mk)

    for l in range(L):
        u = pool.tile([P, 512], F32, tag="u", bufs=2)
        nc.vector.scalar_tensor_tensor(
            out=u[:], in0=Rt[:, fsl], scalar=-2.0 * rs_v[l], in1=rsq[:],
            op0=ALU.mult, op1=ALU.add)
        kp = pool.tile([P, 512], BF16, tag="kp", bufs=2)
        nc.scalar.activation(out=kp[:], in_=u[:], func=AF.Exp,
                             scale=-re_v[l], bias=cb[:, 3 + l:4 + l])
        rsf = pool.tile([P, 512], BF16, tag="rsf", bufs=2)
        nc.vector.scalar_tensor_tensor(
            out=rsf[:], in0=c1s[rcg_of_l[l]][:], scalar=1.0, in1=kp[:],
            op0=ALU.subtract, op1=ALU.mult)
        for a in range(A):
            pm = pool.tile([P, 512], BF16, tag="pm", bufs=2)
            nc.vector.tensor_tensor(out=pm[:], in0=rsf[:], in1=masks[a][:],
                                    op=ALU.mult)
            col0 = 384 * q + a * L + l
            nc.vector.tensor_reduce(
                out=sym[:, col0:col0 + 7 * NFEAT + 1:NFEAT],
                in_=pm[:].rearrange("p (s m) -> p s m", s=8),
                axis=mybir.AxisListType.X, op=ALU.add)

    # ---- batch-norm for this quarter's 384 output columns
    cf = slice(384 * q, 384 * (q + 1))
    ssq = pool.tile([P, 384], F32, tag="ssq", bufs=2)
    nc.vector.tensor_tensor(out=ssq[:], in0=sym[:, cf], in1=sym[:, cf], op=ALU.mult)
    pm1 = psum.tile([8, 384], F32, tag="pbn0")
    nc.tensor.matmul(out=pm1[:], lhsT=bnred[:], rhs=sym[:, cf], start=True, stop=True)
    pm2 = psum.tile([8, 384], F32, tag="pbn1")
    nc.tensor.matmul(out=pm2[:], lhsT=bnred[:], rhs=ssq[:], start=True, stop=True)
    msb = pool.tile([8, 384], F32, tag="msb", bufs=2)
    nc.vector.tensor_copy(out=msb[0:8, :], in_=pm1[:])
    m2 = pool.tile([8, 384], F32, tag="m2", bufs=2)
    nc.vector.tensor_tensor(out=m2[0:8, :], in0=msb[0:8, :], in1=msb[0:8, :],
                            op=ALU.mult)
    vsb = pool.tile([8, 384], F32, tag="vsb", bufs=2)
    nc.vector.tensor_tensor(out=vsb[0:8, :], in0=pm2[:], in1=m2[0:8, :],
                            op=ALU.subtract)
    ssb = pool.tile([8, 384], F32, tag="ssb", bufs=2)
    nc.scalar.activation(out=ssb[0:8, :], in_=vsb[0:8, :], func=AF.Sqrt,
                         bias=cb[0:8, 2:3])
    rsb = pool.tile([8, 384], F32, tag="rsb", bufs=2)
    nc.vector.reciprocal(out=rsb[0:8, :], in_=ssb[0:8, :])
    pbm = psum.tile([P, 384], F32, tag="pbn2")
    nc.tensor.matmul(out=pbm[:], lhsT=bnbc[:], rhs=msb[0:8, :], start=True, stop=True)
    pbr = psum.tile([P, 384], F32, tag="pbn3")
    nc.tensor.matmul(out=pbr[:], lhsT=bnbc[:], rhs=rsb[0:8, :], start=True, stop=True)
    dsb = pool.tile([P, 384], F32, tag="dsb", bufs=2)
    nc.vector.tensor_tensor(out=dsb[:], in0=pbm[:], in1=sym[:, cf], op=ALU.subtract)
    osb = pool.tile([P, 384], F32, tag="osb", bufs=2)
    nc.vector.tensor_tensor(out=osb[:], in0=dsb[:], in1=pbr[:], op=ALU.mult)
    nc.sync.dma_start(out=out_ext[:, cf], in_=osb[:])


# ---------------------------------------------------------------- host side

def make_cbias(rs_v, re_v):
    cb = np.zeros((P, 16), np.float32)
    cb[:, 0] = PI
    cb[:, 1] = 0.5 * PI
    cb[:, 2] = BN_EPS
    cb[:, 15] = 1e-4
    for l in range(L):
        cb[:, 3 + l] = -float(re_v[l]) * float(rs_v[l]) ** 2 + math.log(0.5)
    return cb


def prep_core_inputs(X, Nbrs, Nbrs_Z, r, const_cache={}):
    """Build core r's input map (numpy layout prep only)."""
    n0 = NSH * r
    Xt = np.ascontiguousarray(X.transpose(2, 0, 1))          # [3, B, N]
    if "tbl" not in const_cache:
        tbl = np.zeros((8, 16, 2, N), np.float32)
        tbl[:, 0:3, :, :] = Xt.reshape(3, 8, 2, N).transpose(1, 0, 2, 3)
        const_cache["tbl"] = tbl.reshape(P, 2 * N)

        w1 = np.zeros((P, 8), np.float32)
        wq = np.zeros((P, 8), np.float32)
        for g in range(8):
            w1[16 * g + 0:16 * g + 3, g] = -2.0
            w1[16 * g + 3, g] = 1.0
            wq[16 * g + 0:16 * g + 3, g] = 1.0
        bnred = np.zeros((P, 8), np.float32)
        bnbc = np.zeros((8, P), np.float32)
        for p in range(P):
            bnred[p, p % 8] = 1.0 / 16.0
            bnbc[p % 8, p] = 1.0
        const_cache["w1"] = w1
        const_cache["wq"] = wq
        const_cache["id8"] = np.eye(8, dtype=np.float32)
        const_cache["bnred"] = bnred
        const_cache["bnbc"] = bnbc
        const_cache["cbias"] = None  # filled by caller

    cen = np.zeros((8, 16, 2, NSH), np.float32)
    cen[:, 0:3, :, :] = (Xt[:, :, n0:n0 + NSH]
                         .reshape(3, 8, 2, NSH).transpose(1, 0, 2, 3))
    cen[:, 3, :, :] = 1.0                                     # q_j passthrough
    cen = cen.reshape(P, 2 * NSH)

    nbr_sh = Nbrs[:, n0:n0 + NSH, :]                          # [16, 256, 64]
    nbr6 = nbr_sh.reshape(8, 2, 8, 8, 4, M)                   # [g, beta, nb, k, j, m]
    lg = nbr6 + (np.arange(2, dtype=nbr6.dtype)
                 .reshape(1, 2, 1, 1, 1, 1) * N)
    lg = lg.transpose(0, 3, 1, 2, 4, 5).reshape(8, NCHUNK * CI)
    gidx = (lg.reshape(8, TF, 16).transpose(0, 2, 1)
            .reshape(P, TF).astype(np.int16))

    zc = (Nbrs_Z[:, n0:n0 + NSH, :].reshape(8, 2, 8, 32, M)
          .reshape(P, TF).astype(np.float32))

    return {"tbl": const_cache["tbl"], "gidx": gidx, "cen": cen, "zc": zc,
            "w1": const_cache["w1"], "wq": const_cache["wq"],
            "id8": const_cache["id8"], "bnred": const_cache["bnred"],
            "bnbc": const_cache["bnbc"], "cbias": const_cache["cbias"]}


def assemble_output(results):
    full = np.empty((8, 2, N, NFEAT), np.float32)             # [g, beta, n, f]
    for r in range(8):
        o = np.asarray(results[r]["out"]).reshape(8, 2, NSH, NFEAT)
        n0 = NSH * r
        full[:, :, n0:n0 + NSH, :] = o
    return full.reshape(B, N, NFEAT)


_cache = {}


def kernel(X, Nbrs, Nbrs_Z, rc, rs, re):
    from concourse.bass_utils import run_bass_kernel_spmd
    key = (tuple(np.asarray(rc).ravel().tolist()),
           tuple(np.asarray(rs).ravel().tolist()),
           tuple(np.asarray(re).ravel().tolist()))
    if key not in _cache:
        _cache[key] = build_nc(np.asarray(rc).ravel(), np.asarray(rs).ravel(),
                               np.asarray(re).ravel())
    nc = _cache[key]
    X = np.asarray(X, np.float32)
    Nbrs = np.asarray(Nbrs)
    Nbrs_Z = np.asarray(Nbrs_Z)
    cc = {}
    in_maps = [prep_core_inputs(X, Nbrs, Nbrs_Z, r, cc) for r in range(8)]
    cbias = make_cbias(np.asarray(rs).ravel(), np.asarray(re).ravel())
    for im in in_maps:
        im["cbias"] = cbias
    res = run_bass_kernel_spmd(nc, in_maps, core_ids=list(range(8)))
    return assemble_output(res.results)
